# revision 1
# baseline (speedup 1.0000x reference)
"""Trainium2 Bass kernel for nn_AudioVisualModel loss.

Strategy (8 NeuronCores, data-parallel over audio batch x):
  - Each core owns 3 of the 24 audio batches (150 of 1200 audio tokens),
    and streams the FULL visual matrix (37632 x 768) once from HBM.
  - Per core: normalize audio rows on-chip, normalize visual rows on-chip
    (norms via fused DVE mul+reduce), PE-transpose visual tiles to the
    (d, j) layout (8 blocks batched per PSUM bank, single merged
    evacuation copy), then a bf16 PE matmul produces all token sims for
    this core's audio shard.  The 150 audio rows are zero-padded to 2x128
    so both partition tiles run at M=128 and the whole reduction pipeline
    (max over Nv, min(s,0)^2 sums, temporal diff^2 sums) runs once per
    chunk on merged (128, 2, 392) tiles.
  - Device outputs per core: (24, 3) clip-sim partials and (128, 2)
    per-partition partial sums for the two regularizer terms.  The final
    (24,24) InfoNCE + scalar assembly is done on host (576 elements).
"""

import math
import os
import sys

import numpy as np

sys.path.insert(0, "/opt/trn_rl_repo")

import concourse.bass as bass
import concourse.tile as tile
from concourse import bacc, mybir
from concourse import masks as bass_masks
from concourse.bass_utils import run_bass_kernel_spmd

# Problem shapes (hardcoded per contract).
B, Na, T, Nv, D = 24, 50, 8, 196, 768
NCORES = 8
XPC = B // NCORES              # audio batches per core = 3
AR = XPC * Na                  # audio rows per core = 150
J = B * T * Nv                 # visual rows total = 37632
JY = T * Nv                    # visual rows per y = 1568
NBLK = JY // 128               # full 128-row blocks per y = 12
JREM = JY - NBLK * 128         # remainder rows = 32
KC = D // 128                  # contraction chunks = 6
NCHUNK = 392                   # matmul N chunk = 2 * Nv
CPY = JY // NCHUNK             # chunks per y = 4
EPS = 1e-12

_CACHE = {}


def _build(temp: float, thr: float):
    """Build the Bass module (single SPMD program for all 8 cores)."""
    f32 = mybir.dt.float32
    bf16 = mybir.dt.bfloat16

    nc = bacc.Bacc(
        "TRN2",
        target_bir_lowering=False,
        debug=False,
        enable_asserts=False,
        num_devices=NCORES,
    )

    a_in = nc.dram_tensor("a", [AR, D], f32, kind="ExternalInput").ap()
    v_in = nc.dram_tensor("v", [J, D], f32, kind="ExternalInput").ap()
    ind_in = nc.dram_tensor("ind", [128, 2 * XPC], f32, kind="ExternalInput").ap()
    clip_out = nc.dram_tensor("clip", [B, XPC], f32, kind="ExternalOutput").ap()
    acc_out = nc.dram_tensor("acc", [128, 2], f32, kind="ExternalOutput").ap()

    MT = [(0, 128), (1, AR - 128)]  # audio partition tiles (index, valid rows)

    with tile.TileContext(nc) as tc:
        from contextlib import ExitStack

        ctx = ExitStack()
        with ctx:
            singles = ctx.enter_context(tc.tile_pool(name="singles", bufs=1))
            vpool = ctx.enter_context(tc.tile_pool(name="vload", bufs=2))
            vtpool = ctx.enter_context(tc.tile_pool(name="vt", bufs=2))
            scrpool = ctx.enter_context(tc.tile_pool(name="scr", bufs=2))
            smpool = ctx.enter_context(tc.tile_pool(name="sm", bufs=2))
            tiny = ctx.enter_context(tc.tile_pool(name="tiny", bufs=3))
            mmpool = ctx.enter_context(
                tc.tile_pool(name="mm", bufs=2, space="PSUM")
            )
            tppool = ctx.enter_context(
                tc.tile_pool(name="tp", bufs=3, space="PSUM")
            )
            clpool = ctx.enter_context(
                tc.tile_pool(name="cl", bufs=1, space="PSUM")
            )

            ident = singles.tile([128, 128], bf16)
            bass_masks.make_identity(nc, ident[:])

            indt = singles.tile([128, 2 * XPC], f32)
            nc.sync.dma_start(out=indt[:], in_=ind_in)

            # ---------------- audio prep ----------------
            # aT[k] = k-th 128-row d-chunk of normalized-audio^T, zero-padded
            # to 256 audio columns so both M-tiles run at M=128.
            aT = singles.tile([128, KC, 256], bf16)
            nc.vector.memset(aT[:], 0.0)
            for mi, M in MT:
                at = tiny.tile([128, D], f32, tag="aload", name="at")
                nc.sync.dma_start(out=at[:M], in_=a_in[mi * 128 : mi * 128 + M, :])
                scr = tiny.tile([128, D], f32, tag="ascr", name="scr")
                n2 = tiny.tile([128, 1], f32, tag="an2", name="n2")
                nc.vector.affine_mul_reduce(
                    out=scr[:M],
                    accum_out=n2[:M],
                    in0=at[:M],
                    in1=at[:M],
                    scale=1.0,
                    bias=0.0,
                )
                nrm = tiny.tile([128, 1], f32, tag="anrm", name="nrm")
                nc.scalar.activation(
                    nrm[:M], n2[:M], mybir.ActivationFunctionType.Sqrt
                )
                nc.vector.tensor_scalar_max(nrm[:M], nrm[:M], EPS)
                rn = tiny.tile([128, 1], f32, tag="arn", name="rn")
                nc.vector.reciprocal(rn[:M], nrm[:M])
                ab = tiny.tile([128, D], bf16, tag="ab", name="ab")
                nc.vector.tensor_scalar_mul(ab[:M], at[:M], rn[:M])
                for k in range(KC):
                    pt = tppool.tile([128, 1024], bf16, tag="tp", name="pta")
                    nc.tensor.transpose(
                        pt[:, :M],
                        ab[:M, k * 128 : (k + 1) * 128],
                        ident[:M, :M],
                    )
                    if k % 2 == 0:
                        nc.vector.tensor_copy(
                            aT[:, k, mi * 128 : mi * 128 + M], pt[:, :M]
                        )
                    else:
                        nc.scalar.copy(
                            aT[:, k, mi * 128 : mi * 128 + M], pt[:, :M]
                        )

            # accumulator columns (one per y), merged across both M-tiles
            maxv = singles.tile([128, 2, B * T], f32)
            nncol = singles.tile([128, B], f32)
            tdcol = singles.tile([128, B], f32)

            # transpose evac groups per k: blocks [0..8) and [8..13)
            GROUPS = [(0, 8, 1024), (8, 5, 544)]

            # ---------------- visual sweep ----------------
            for y in range(B):
                vb = vpool.tile([128, NBLK + 1, D], bf16, tag="vb", name="vb")
                src = v_in[y * JY : y * JY + NBLK * 128, :].rearrange(
                    "(b p) d -> p b d", p=128
                )
                nc.gpsimd.dma_start(out=vb[:, :NBLK, :], in_=src)
                nc.gpsimd.dma_start(
                    out=vb[:JREM, NBLK, :],
                    in_=v_in[y * JY + NBLK * 128 : (y + 1) * JY, :],
                )

                # row norms -> rnv (128, NBLK+1)
                n2c = tiny.tile([128, NBLK + 1], f32, tag="n2c", name="n2c")
                for b in range(NBLK + 1):
                    P = 128 if b < NBLK else JREM
                    scrv = scrpool.tile([128, D], bf16, tag="scrv", name="scrv")
                    if b < 6:
                        nc.vector.affine_mul_reduce(
                            out=scrv[:P],
                            accum_out=n2c[:P, b : b + 1],
                            in0=vb[:P, b, :],
                            in1=vb[:P, b, :],
                            scale=1.0,
                            bias=0.0,
                        )
                    else:
                        nc.scalar.activation(
                            scrv[:P],
                            vb[:P, b, :],
                            mybir.ActivationFunctionType.Square,
                            accum_out=n2c[:P, b : b + 1],
                        )
                nrmv = tiny.tile([128, NBLK + 1], f32, tag="nrmv", name="nrmv")
                # sqrt(n2 * temp^2) = ||v|| * temp
                nc.scalar.activation(
                    nrmv[:],
                    n2c[:],
                    mybir.ActivationFunctionType.Sqrt,
                    scale=float(temp * temp),
                )
                nc.vector.tensor_scalar_max(nrmv[:], nrmv[:], EPS)
                rnv = tiny.tile([128, NBLK + 1], f32, tag="rnv", name="rnv")
                nc.vector.reciprocal(rnv[:], nrmv[:])

                # normalize rows in place (bf16)
                for b in range(NBLK + 1):
                    P = 128 if b < NBLK else JREM
                    nc.vector.tensor_scalar_mul(
                        vb[:P, b, :], vb[:P, b, :], rnv[:P, b : b + 1]
                    )

                # transpose to vt (128, KC, JY); 8 blocks share one PSUM bank
                vt = vtpool.tile([128, KC, JY], bf16, tag="vt", name="vt")
                ei = 0
                for k in range(KC):
                    for b0, nb, width in GROUPS:
                        pt = tppool.tile([128, 1024], bf16, tag="tp", name="ptv")
                        for i in range(nb):
                            b = b0 + i
                            P = 128 if b < NBLK else JREM
                            nc.tensor.transpose(
                                pt[:, i * 128 : i * 128 + P],
                                vb[:P, b, k * 128 : (k + 1) * 128],
                                ident[:P, :P],
                            )
                        dst = vt[:, k, b0 * 128 : b0 * 128 + width]
                        if ei % 2 == 0:
                            nc.vector.tensor_copy(dst, pt[:, :width])
                        else:
                            nc.scalar.copy(dst, pt[:, :width])
                        ei += 1

                # main matmul + fused reductions (both M-tiles in one tile)
                s_sb = smpool.tile([128, 2, JY], bf16, tag="s", name="s_sb")
                m_y = smpool.tile([128, 2, JY], bf16, tag="m", name="m_y")
                dif_y = smpool.tile(
                    [128, 2, (T - 1) * Nv], bf16, tag="dif", name="dif_y"
                )
                for c in range(CPY):
                    # mi stride padded to one full PSUM bank (512 f32)
                    psfull = mmpool.tile([128, 2, 512], f32, tag="ps", name="ps")
                    ps = psfull[:, :, :NCHUNK]
                    for mi, M in MT:
                        for k in range(KC):
                            nc.tensor.matmul(
                                ps[:, mi, :],
                                lhsT=aT[:, k, mi * 128 : (mi + 1) * 128],
                                rhs=vt[:, k, c * NCHUNK : (c + 1) * NCHUNK],
                                start=(k == 0),
                                stop=(k == KC - 1),
                            )
                    # stage sims to SBUF (bf16) in one copy
                    nc.scalar.copy(
                        s_sb[:, :, c * NCHUNK : (c + 1) * NCHUNK], ps[:]
                    )
                    # max over Nv for the two t-groups (both M-tiles)
                    nc.vector.reduce_max(
                        maxv[:, :, y * T + 2 * c : y * T + 2 * c + 2],
                        ps[:].rearrange("p m (t v) -> p m t v", v=Nv),
                        axis=mybir.AxisListType.X,
                    )
                    # clip(s, -20, 0) from staged sims (bf16 fast path)
                    nc.vector.tensor_scalar(
                        out=m_y[:, :, c * NCHUNK : (c + 1) * NCHUNK],
                        in0=s_sb[:, :, c * NCHUNK : (c + 1) * NCHUNK],
                        scalar1=0.0,
                        scalar2=-20.0,
                        op0=mybir.AluOpType.min,
                        op1=mybir.AluOpType.max,
                    )
                # temporal diffs from the staged SBUF sims
                sv = s_sb.rearrange("p m (t v) -> p m t v", v=Nv)
                dv = dif_y.rearrange("p m (t v) -> p m t v", v=Nv)
                for t in range(T - 1):
                    nc.gpsimd.tensor_tensor(
                        out=dv[:, :, t, :],
                        in0=sv[:, :, t + 1, :],
                        in1=sv[:, :, t, :],
                        op=mybir.AluOpType.subtract,
                    )
                scrm = scrpool.tile([128, 2, JY], bf16, tag="scrm", name="scrm")
                nc.scalar.activation(
                    scrm[:],
                    m_y[:],
                    mybir.ActivationFunctionType.Square,
                    accum_out=nncol[:, y : y + 1],
                )
                scrd = scrpool.tile(
                    [128, 2, (T - 1) * Nv], bf16, tag="scrd", name="scrd"
                )
                nc.scalar.activation(
                    scrd[:],
                    dif_y[:],
                    mybir.ActivationFunctionType.Square,
                    accum_out=tdcol[:, y : y + 1],
                )

            # ---------------- epilogue ----------------
            mask = tiny.tile([128, 2, B * T], f32, tag="mask", name="mask")
            nc.vector.tensor_scalar(
                out=mask[:],
                in0=maxv[:],
                scalar1=thr,
                scalar2=None,
                op0=mybir.AluOpType.is_ge,
            )
            msked = tiny.tile([128, 2, B * T], f32, tag="msk", name="msked")
            nc.vector.tensor_tensor(
                out=msked[:], in0=maxv[:], in1=mask[:], op=mybir.AluOpType.mult
            )
            counts = tiny.tile([128, 2, B], f32, tag="cnt", name="counts")
            nc.vector.reduce_sum(
                counts[:],
                mask.rearrange("p m (y t) -> p m y t", t=T),
                axis=mybir.AxisListType.X,
            )
            toksum = tiny.tile([128, 2, B], f32, tag="tks", name="toksum")
            nc.vector.reduce_sum(
                toksum[:],
                msked.rearrange("p m (y t) -> p m y t", t=T),
                axis=mybir.AxisListType.X,
            )
            nc.vector.tensor_scalar_max(counts[:], counts[:], 1.0)
            rcc = tiny.tile([128, 2, B], f32, tag="rcc", name="rcc")
            nc.vector.reciprocal(rcc[:], counts[:])
            tok = tiny.tile([128, 2, B], f32, tag="tok", name="tok")
            nc.vector.tensor_tensor(
                out=tok[:], in0=toksum[:], in1=rcc[:], op=mybir.AluOpType.mult
            )
            # mean over audio tokens within each local x: ones-matmul
            psc = clpool.tile([B, XPC], f32, name="psc")
            for mi, M in MT:
                nc.tensor.matmul(
                    psc[:, :],
                    lhsT=tok[:, mi, :],
                    rhs=indt[:, mi * XPC : (mi + 1) * XPC],
                    start=(mi == 0),
                    stop=(mi == 1),
                )
            # regularizer partials
            accs = tiny.tile([128, 2], f32, tag="accs", name="accs")
            nc.vector.reduce_sum(
                accs[:, 0:1], nncol[:], axis=mybir.AxisListType.X
            )
            nc.vector.reduce_sum(
                accs[:, 1:2], tdcol[:], axis=mybir.AxisListType.X
            )
            nc.sync.dma_start(out=acc_out[:, :], in_=accs[:])
            cls = tiny.tile([B, XPC], f32, tag="cls", name="cls")
            nc.vector.tensor_copy(cls[:], psc[:])
            nc.sync.dma_start(out=clip_out[:, :], in_=cls[:])

    nc.compile()
    return nc


def _make_ind():
    ind = np.zeros((128, 2 * XPC), dtype=np.float32)
    for mi in range(2):
        for p in range(128):
            row = mi * 128 + p
            if row < AR:
                g = row // Na
                ind[p, mi * XPC + g] = 1.0 / Na
    return ind


def kernel(audio_feats, visual_feats, temperature, threshold):
    temp = float(np.asarray(temperature))
    thr_in = float(np.asarray(threshold))
    thr = 1.0 / (1.0 + math.exp(-thr_in))  # sigmoid

    key = (temp, thr_in)
    if key not in _CACHE:
        _CACHE[key] = _build(temp, thr)
    nc = _CACHE[key]

    a = np.ascontiguousarray(
        np.asarray(audio_feats, dtype=np.float32).reshape(B * Na, D)
    )
    v = np.ascontiguousarray(
        np.asarray(visual_feats, dtype=np.float32).reshape(J, D)
    )
    ind = _make_ind()

    in_maps = []
    for c in range(NCORES):
        in_maps.append({"a": a[c * AR : (c + 1) * AR], "v": v, "ind": ind})

    res = run_bass_kernel_spmd(nc, in_maps, core_ids=list(range(NCORES)))
    outs = res.results

    # host assembly (576-element InfoNCE + scalar reg terms)
    clip = np.zeros((B, B), dtype=np.float64)
    s_nonneg = 0.0
    s_tdiff = 0.0
    for c in range(NCORES):
        co = outs[c]["clip"].astype(np.float64)  # (B=y, XPC=g)
        for g in range(XPC):
            clip[c * XPC + g, :] = co[:, g]
        acc = outs[c]["acc"].astype(np.float64)  # (128, 2)
        s_nonneg += acc[:, 0].sum()
        s_tdiff += acc[:, 1].sum()

    def logsumexp(m, axis):
        mx = m.max(axis=axis, keepdims=True)
        return mx + np.log(np.exp(m - mx).sum(axis=axis, keepdims=True))

    diag = np.arange(B)
    lsm1 = clip - logsumexp(clip, 1)
    lsm0 = clip - logsumexp(clip, 0)
    contrastive = -(lsm1[diag, diag] + lsm0[diag, diag]).mean() / 2.0

    l_nonneg = s_nonneg / (B * B * Na * T * Nv)
    l_temporal = s_tdiff / (B * B * Na * (T - 1) * Nv)
    log_t = math.log(temp)
    temp_low = max(math.log(2.3) - log_t, 0.0) ** 3
    temp_high = max(log_t - math.log(4.0), 0.0) ** 3
    reg = 0.15 * l_nonneg + 8.0 * (temp_low + temp_high) + 0.01 * l_temporal

    return np.float32(contrastive + reg)



# revision 22
# speedup vs baseline: 2.5503x; 2.5503x over previous
"""Trainium2 Bass kernel for nn_AudioVisualModel loss.

Strategy (8 NeuronCores, data-parallel over the VISUAL batch y):
  - Each core owns 3 of the 24 visual batches (4704 of 37632 visual rows,
    14.5MB) and replicates the full audio matrix (1200 rows, 3.7MB) --
    ~18MB HBM traffic per core vs ~119MB for audio-sharding.
  - Audio rows pad 1200->1280 (10 M-tiles of 128, 6.7% pad waste).
  - bf16 PE matmul produces sims per (M-tile, 1568-col group) in PSUM;
    stage copies to a SBUF bf16 slab rotate over act/gpsimd/vector.
  - Reductions exploit DVE fast modes (tensor_scalar 4x, tensor_tensor
    2x; the fused scalar_tensor_tensor runs 1x so it is avoided on the
    hot path):
      sq   = s*s                      (tt 2x)
      S_all, edge sums = ts+accum(sq) (4x)
      prod = s[t+1]*s[t]              (tt 2x, shifted views, per y)
      S_cross = ts+accum(prod)        (4x)
      mneg = min(s,0)                 (ts 4x)
      S_nonneg = Square(mneg)+accum   (act)
      max over Nv: tt-max halvings 196->98->49 (2x) + 49-wide reduce
    temporal loss = 2*S_all - S_t0 - S_t7 - 2*S_cross (exact identity;
    the reference's clamp at -20 is a no-op when 1/temp <= 20 -- slow
    path otherwise).
  - Visual norms on gpsimd, audio norms/muls on act, visual muls DVE 4x.
  - Device outputs per core: (3, 24) clip-sim partials and (128, 10, 8)
    per-partition reg partial sums. Host does the 576-element InfoNCE
    and final scalar assembly in f64.
"""

import math
import sys

import numpy as np

sys.path.insert(0, "/opt/trn_rl_repo")

import concourse.bass as bass
import concourse.tile as tile
from concourse import bacc, mybir
from concourse import masks as bass_masks
from concourse.bass_utils import run_bass_kernel_spmd

# Problem shapes (hardcoded per contract).
B, Na, T, Nv, D = 24, 50, 8, 196, 768
NCORES = 8
YPC = B // NCORES               # visual batches per core = 3
JY = T * Nv                     # visual rows per y = 1568
JC = YPC * JY                   # visual rows per core = 4704
AR = B * Na                     # audio rows total = 1200
MT = 10                         # audio M-tiles (1280 padded)
ABLK = 10                       # audio 128-blocks (9 full + 48)
VBLK = 37                       # visual 128-blocks (36 full + 96)
KC = D // 128                   # contraction chunks = 6
NCH = 2 * Nv                    # matmul N chunk = 392 (fits a PSUM bank)
NGRP = 3                        # chunk groups per M-tile (4 chunks each)
ACC_K = 8                       # accumulator slots per M-tile
EPS = 1e-12

_CACHE = {}


def _build(temp: float, thr: float):
    f32 = mybir.dt.float32
    bf16 = mybir.dt.bfloat16
    Alu = mybir.AluOpType
    Act = mybir.ActivationFunctionType

    fast_nonneg = (1.0 / temp) <= 20.0

    nc = bacc.Bacc(
        "TRN2",
        target_bir_lowering=False,
        debug=False,
        enable_asserts=False,
        num_devices=NCORES,
    )

    a_in = nc.dram_tensor("a", [AR, D], f32, kind="ExternalInput").ap()
    v_in = nc.dram_tensor("v", [JC, D], f32, kind="ExternalInput").ap()
    ind_in = nc.dram_tensor("ind", [128, MT * B], f32, kind="ExternalInput").ap()
    clip_out = nc.dram_tensor("clip", [YPC, B], f32, kind="ExternalOutput").ap()
    acc_out = nc.dram_tensor("acc", [128, MT * ACC_K], f32, kind="ExternalOutput").ap()

    with tile.TileContext(nc) as tc:
        from contextlib import ExitStack

        ctx = ExitStack()
        with ctx:
            singles = ctx.enter_context(tc.tile_pool(name="singles", bufs=1))
            tiny = ctx.enter_context(tc.tile_pool(name="tiny", bufs=2))

            ident = singles.tile([128, 128], bf16)
            bass_masks.make_identity(nc, ident[:])

            indt = singles.tile([128, MT, B], f32)
            nc.sync.dma_start(out=indt[:], in_=ind_in)

            acc = singles.tile([128, MT, ACC_K], f32)
            nc.vector.memset(acc[:], 0.0)
            maxv_all = singles.tile([128, MT, YPC * T], f32)

            # aT: 16*normalized-audio^T as fp8 pairs (128, KC/2, 2, 1280);
            # vt: 16*visual^T/temp likewise. The PE matmul runs fp8
            # DoubleRow (2 k-chunks per instruction, 0.5 cycles/row); the
            # stage copies undo the 16*16 scaling.
            fp8 = mybir.dt.float8e4
            aT = singles.tile([128, KC // 2, 2, MT * 128], fp8)
            nc.vector.memset(aT[:, :, :, AR:], 0.0)
            vt = singles.tile([128, KC // 2, 2, JC], fp8)

            # ---------------- prep phase ----------------
            # Visual groups stream first (Pool/DVE norms alternate per
            # block); audio (bf16, tiny) is interleaved after group 1 so
            # aT is ready by the time the first matmuls can start.
            with ExitStack() as pctx:
                apool = pctx.enter_context(tc.tile_pool(name="ap", bufs=1))
                vbpool = pctx.enter_context(tc.tile_pool(name="vb", bufs=3))
                ptiny = pctx.enter_context(tc.tile_pool(name="pt", bufs=2))
                tppool = pctx.enter_context(
                    tc.tile_pool(name="tp", bufs=3, space="PSUM")
                )

                at = apool.tile([128, ABLK, D], bf16)
                ab = apool.tile([128, ABLK, D], bf16)
                n2a = ptiny.tile([128, ABLK], f32, tag="n2a", name="n2a")
                nrma = ptiny.tile([128, ABLK], f32, tag="nrma", name="nrma")
                rna = ptiny.tile([128, ABLK], f32, tag="rna", name="rna")
                n2v = ptiny.tile([128, VBLK], f32, tag="n2v", name="n2v")
                rnv = ptiny.tile([128, VBLK], f32, tag="rnv", name="rnv")
                ascr = apool.tile([128, D], bf16)
                nc.vector.memset(n2a[:], 1.0)
                nc.vector.memset(n2v[:], 1.0)

                def audio_prep():
                    nc.gpsimd.dma_start(
                        out=at[:, :9, :],
                        in_=a_in[: 9 * 128, :].rearrange("(b p) d -> p b d", p=128),
                    )
                    nc.gpsimd.dma_start(out=at[:48, 9, :], in_=a_in[9 * 128 :, :])
                    for b in range(ABLK):
                        P = 128 if b < 9 else 48
                        nc.vector.affine_mul_reduce(
                            out=ascr[:P],
                            accum_out=n2a[:P, b : b + 1],
                            in0=at[:P, b, :],
                            in1=at[:P, b, :],
                            scale=1.0,
                            bias=0.0,
                        )
                    # sqrt(n2/256) = ||a||/16  (fp8 operand scale)
                    nc.scalar.activation(
                        nrma[:], n2a[:], Act.Sqrt, scale=1.0 / 256.0
                    )
                    nc.vector.tensor_scalar_max(nrma[:], nrma[:], EPS)
                    nc.vector.reciprocal(rna[:], nrma[:])
                    for b in range(ABLK):
                        P = 128 if b < 9 else 48
                        nc.vector.tensor_scalar_mul(
                            ab[:P, b, :], at[:P, b, :], rna[:P, b : b + 1]
                        )
                    for k in range(KC):
                        for g0, nb in ((0, 8), (8, 2)):
                            pt = tppool.tile([128, 1024], bf16, tag="tp", name="pta")
                            for i in range(nb):
                                b = g0 + i
                                P = 128 if b < 9 else 48
                                nc.tensor.transpose(
                                    pt[:, i * 128 : i * 128 + P],
                                    ab[:P, b, k * 128 : (k + 1) * 128],
                                    ident[:P, :P],
                                )
                            width = (
                                nb * 128 if g0 + nb < ABLK else (nb - 1) * 128 + 48
                            )
                            dst = aT[:, k // 2, k % 2, g0 * 128 : g0 * 128 + width]
                            if k % 2 == 0:
                                nc.vector.tensor_copy(dst, pt[:, :width])
                            else:
                                nc.scalar.copy(dst, pt[:, :width])

                ei = 0
                for gi, (g0, nb) in enumerate(
                    ((0, 8), (8, 8), (16, 8), (24, 8), (32, 5))
                ):
                    vb = vbpool.tile([128, 8, D], bf16, tag="vb", name="vb")
                    nfull = nb if g0 + nb < VBLK else nb - 1
                    src = v_in[g0 * 128 : (g0 + nfull) * 128, :].rearrange(
                        "(b p) d -> p b d", p=128
                    )
                    nc.gpsimd.dma_start(out=vb[:, :nfull, :], in_=src)
                    if g0 + nb == VBLK:
                        nc.gpsimd.dma_start(
                            out=vb[:96, nb - 1, :],
                            in_=v_in[(g0 + nfull) * 128 :, :],
                        )
                    vscr = ptiny.tile([128, D], bf16, tag="vscr", name="vscr")
                    for i in range(nb):
                        b = g0 + i
                        P = 128 if b < VBLK - 1 else 96
                        nc.vector.affine_mul_reduce(
                            out=vscr[:P],
                            accum_out=n2v[:P, b : b + 1],
                            in0=vb[:P, i, :],
                            in1=vb[:P, i, :],
                            scale=1.0,
                            bias=0.0,
                        )
                    # sqrt(n2 * temp^2/256) = ||v||*temp/16 (fp8 scale)
                    nrmv = ptiny.tile([128, 8], f32, tag="nrmv", name="nrmv")
                    nc.scalar.activation(
                        nrmv[:, :nb],
                        n2v[:, g0 : g0 + nb],
                        Act.Sqrt,
                        scale=float(temp * temp) / 256.0,
                    )
                    nc.vector.tensor_scalar_max(nrmv[:, :nb], nrmv[:, :nb], EPS)
                    nc.vector.reciprocal(rnv[:, g0 : g0 + nb], nrmv[:, :nb])
                    for i in range(nb):
                        b = g0 + i
                        P = 128 if b < VBLK - 1 else 96
                        nc.vector.tensor_scalar_mul(
                            vb[:P, i, :], vb[:P, i, :], rnv[:P, b : b + 1]
                        )
                    for k in range(KC):
                        pt = tppool.tile([128, 1024], bf16, tag="tp", name="ptv")
                        for i in range(nb):
                            b = g0 + i
                            P = 128 if b < VBLK - 1 else 96
                            nc.tensor.transpose(
                                pt[:, i * 128 : i * 128 + P],
                                vb[:P, i, k * 128 : (k + 1) * 128],
                                ident[:P, :P],
                            )
                        width = nb * 128 if g0 + nb < VBLK else (nb - 1) * 128 + 96
                        dst = vt[:, k // 2, k % 2, g0 * 128 : g0 * 128 + width]
                        if ei % 2 == 0:
                            nc.vector.tensor_copy(dst, pt[:, :width])
                        else:
                            nc.scalar.copy(dst, pt[:, :width])
                        ei += 1
                    if gi == 1:
                        audio_prep()

            # ---------------- main m-loop ----------------
            with ExitStack() as mctx:
                slabpool = mctx.enter_context(tc.tile_pool(name="slab", bufs=2))
                scrpool = mctx.enter_context(tc.tile_pool(name="scr", bufs=2))
                mnpool = mctx.enter_context(tc.tile_pool(name="mn", bufs=2))
                hpool = mctx.enter_context(tc.tile_pool(name="h", bufs=2))
                mmpool = mctx.enter_context(
                    tc.tile_pool(name="mm", bufs=2, space="PSUM")
                )
                for m in range(MT):
                    # slab holds 256*s (fp8 operand scale); all rescaling
                    # is folded into host sums / ind / the thr constant.
                    slab = slabpool.tile([128, JC], bf16, tag="s", name="slab")
                    for g in range(NGRP):
                        ps = mmpool.tile([128, 4, 512], f32, tag="ps", name="ps")
                        for kk in range(KC // 2):
                            for c in range(4):
                                j0 = (g * 4 + c) * NCH
                                nc.tensor.matmul(
                                    ps[:, c, :NCH],
                                    lhsT=aT[:, kk, :, m * 128 : (m + 1) * 128],
                                    rhs=vt[:, kk, :, j0 : j0 + NCH],
                                    start=(kk == 0),
                                    stop=(kk == KC // 2 - 1),
                                    perf_mode=mybir.MatmulPerfMode.DoubleRow,
                                )
                        dst = slab[:, g * 4 * NCH : (g + 1) * 4 * NCH].rearrange(
                            "p (c j) -> p c j", c=4
                        )
                        src_ps = ps[:, :, :NCH]
                        if g == 1:
                            nc.vector.tensor_copy(dst, src_ps)
                        else:
                            nc.scalar.copy(dst, src_ps)

                    sy = slab.rearrange("p (y c) -> p y c", y=YPC)
                    scr = scrpool.tile([128, JC], bf16, tag="stt", name="sttscr")
                    mscr = scrpool.tile([128, JC], bf16, tag="ms", name="mscr")
                    # sum s^2: act for most M-tiles, DVE (tt 2x + ts 4x) for
                    # the rest to balance engine load
                    nc.scalar.activation(
                        mscr[:],
                        slab[:],
                        Act.Square,
                        accum_out=acc[:, m, 0:1],
                    )
                    # edge sums (DVE affine_mul_reduce, small)
                    vsc2 = scrpool.tile([128, YPC, Nv], bf16, tag="e", name="escr")
                    nc.vector.affine_mul_reduce(
                        out=vsc2[:],
                        accum_out=acc[:, m, 3:4],
                        in0=sy[:, :, :Nv],
                        in1=sy[:, :, :Nv],
                        scale=1.0,
                        bias=0.0,
                    )
                    nc.vector.affine_mul_reduce(
                        out=vsc2[:],
                        accum_out=acc[:, m, 4:5],
                        in0=sy[:, :, JY - Nv :],
                        in1=sy[:, :, JY - Nv :],
                        scale=1.0,
                        bias=0.0,
                    )
                    # temporal cross term: prod (tt 2x) then 4x in-place accum
                    prod = scr[:, : YPC * (JY - Nv)].rearrange(
                        "p (y c) -> p y c", y=YPC
                    )
                    nc.gpsimd.tensor_tensor(
                        out=prod,
                        in0=sy[:, :, Nv:],
                        in1=sy[:, :, : JY - Nv],
                        op=Alu.mult,
                    )
                    nc.vector.tensor_scalar(
                        out=prod,
                        in0=prod,
                        scalar1=0.0,
                        scalar2=0.0,
                        op0=Alu.add,
                        op1=Alu.add,
                        accum_out=acc[:, m, 2:3],
                    )
                    # nonneg: mneg = min(s,0) (ts 4x), square+accum on act
                    mneg = mnpool.tile([128, JC], bf16, tag="mn", name="mneg")
                    if fast_nonneg:
                        nc.vector.tensor_scalar_min(mneg[:], slab[:], 0.0)
                    else:
                        nc.vector.tensor_scalar(
                            out=mneg[:],
                            in0=slab[:],
                            scalar1=0.0,
                            scalar2=-20.0,
                            op0=Alu.min,
                            op1=Alu.max,
                        )
                    nc.scalar.activation(
                        mscr[:],
                        mneg[:],
                        Act.Square,
                        accum_out=acc[:, m, 1:2],
                    )
                    # max over Nv: two tt-max halvings then a 49-wide reduce
                    sg = slab.rearrange("p (g v) -> p g v", v=Nv)  # g = y*T+t
                    h1 = hpool.tile([128, YPC * T, 98], bf16, tag="h1", name="h1")
                    nc.vector.tensor_tensor(
                        out=h1[:], in0=sg[:, :, :98], in1=sg[:, :, 98:], op=Alu.max
                    )
                    h2 = hpool.tile([128, YPC * T, 49], bf16, tag="h2", name="h2")
                    nc.vector.tensor_tensor(
                        out=h2[:], in0=h1[:, :, :49], in1=h1[:, :, 49:], op=Alu.max
                    )
                    nc.vector.reduce_max(
                        maxv_all[:, m, :],
                        h2[:],
                        axis=mybir.AxisListType.X,
                    )

            # ---------------- epilogue ----------------
            with ExitStack() as ectx:
                clpool = ectx.enter_context(
                    tc.tile_pool(name="cl", bufs=1, space="PSUM")
                )
                epool = ectx.enter_context(tc.tile_pool(name="ep", bufs=1))
                mask = epool.tile([128, MT, YPC, T], f32)
                # maxv is at 256x scale; compare against 256*thr
                nc.vector.tensor_scalar(
                    out=mask.rearrange("p m y t -> p (m y t)"),
                    in0=maxv_all.rearrange("p m g -> p (m g)"),
                    scalar1=thr * 256.0,
                    scalar2=None,
                    op0=Alu.is_ge,
                )
                msked = epool.tile([128, MT, YPC, T], f32)
                nc.vector.tensor_tensor(
                    out=msked.rearrange("p m y t -> p (m y t)"),
                    in0=maxv_all.rearrange("p m g -> p (m g)"),
                    in1=mask.rearrange("p m y t -> p (m y t)"),
                    op=Alu.mult,
                )
                counts = epool.tile([128, MT, YPC], f32, tag="cnt", name="counts")
                nc.vector.reduce_sum(counts[:], mask[:], axis=mybir.AxisListType.X)
                toksum = epool.tile([128, MT, YPC], f32, tag="tks", name="toksum")
                nc.vector.reduce_sum(toksum[:], msked[:], axis=mybir.AxisListType.X)
                nc.vector.tensor_scalar_max(counts[:], counts[:], 1.0)
                rcc = epool.tile([128, MT, YPC], f32, tag="rcc", name="rcc")
                nc.vector.reciprocal(rcc[:], counts[:])
                tok = epool.tile([128, MT, YPC], f32, tag="tok", name="tok")
                nc.vector.tensor_tensor(
                    out=tok.rearrange("p m y -> p (m y)"),
                    in0=toksum.rearrange("p m y -> p (m y)"),
                    in1=rcc.rearrange("p m y -> p (m y)"),
                    op=Alu.mult,
                )
                psc = clpool.tile([YPC, B], f32, name="psc")
                for m in range(MT):
                    nc.tensor.matmul(
                        psc[:, :],
                        lhsT=tok[:, m, :],
                        rhs=indt[:, m, :],
                        start=(m == 0),
                        stop=(m == MT - 1),
                    )
                cls = epool.tile([YPC, B], f32, tag="cls", name="cls")
                nc.vector.tensor_copy(cls[:], psc[:])
                nc.sync.dma_start(out=clip_out[:, :], in_=cls[:])
                nc.sync.dma_start(
                    out=acc_out[:, :], in_=acc.rearrange("p m k -> p (m k)")
                )

    nc.compile()
    return nc


def _make_ind():
    # 1/(Na*256): folds the fp8 256x operand scale out of the clip sims
    ind = np.zeros((128, MT, B), dtype=np.float32)
    for m in range(MT):
        for p in range(128):
            row = m * 128 + p
            if row < AR:
                ind[p, m, row // Na] = 1.0 / (Na * 256.0)
    return ind.reshape(128, MT * B)


def kernel(audio_feats, visual_feats, temperature, threshold):
    temp = float(np.asarray(temperature))
    thr_in = float(np.asarray(threshold))
    thr = 1.0 / (1.0 + math.exp(-thr_in))  # sigmoid

    key = (temp, thr_in)
    if key not in _CACHE:
        _CACHE[key] = _build(temp, thr)
    nc = _CACHE[key]

    a = np.ascontiguousarray(
        np.asarray(audio_feats, dtype=np.float32).reshape(AR, D)
    )
    v = np.asarray(visual_feats, dtype=np.float32).reshape(B * JY, D)
    ind = _make_ind()

    in_maps = []
    for c in range(NCORES):
        in_maps.append(
            {
                "a": a,
                "v": np.ascontiguousarray(v[c * JC : (c + 1) * JC]),
                "ind": ind,
            }
        )

    res = run_bass_kernel_spmd(nc, in_maps, core_ids=list(range(NCORES)))
    outs = res.results

    clip = np.zeros((B, B), dtype=np.float64)
    s_all = s_nn = s_cr = s_e0 = s_e7 = 0.0
    for c in range(NCORES):
        co = outs[c]["clip"].astype(np.float64)  # (YPC, B): [y_local, x]
        for yl in range(YPC):
            clip[:, c * YPC + yl] = co[yl, :]
        # device sums are at (256*s)^2 scale
        ac = outs[c]["acc"].astype(np.float64).reshape(128, MT, ACC_K) / 65536.0
        s_all += ac[:, :, 0].sum()
        s_nn += ac[:, :, 1].sum()
        s_cr += ac[:, :, 2].sum()
        s_e0 += ac[:, :, 3].sum()
        s_e7 += ac[:, :, 4].sum()

    def logsumexp(m, axis):
        mx = m.max(axis=axis, keepdims=True)
        return mx + np.log(np.exp(m - mx).sum(axis=axis, keepdims=True))

    diag = np.arange(B)
    lsm1 = clip - logsumexp(clip, 1)
    lsm0 = clip - logsumexp(clip, 0)
    contrastive = -(lsm1[diag, diag] + lsm0[diag, diag]).mean() / 2.0

    l_nonneg = s_nn / (B * B * Na * T * Nv)
    td_sum = 2.0 * s_all - s_e0 - s_e7 - 2.0 * s_cr
    l_temporal = td_sum / (B * B * Na * (T - 1) * Nv)
    log_t = math.log(temp)
    temp_low = max(math.log(2.3) - log_t, 0.0) ** 3
    temp_high = max(log_t - math.log(4.0), 0.0) ** 3
    reg = 0.15 * l_nonneg + 8.0 * (temp_low + temp_high) + 0.01 * l_temporal

    return np.float32(contrastive + reg)


# revision 25
# speedup vs baseline: 2.6756x; 1.0491x over previous
"""Trainium2 Bass kernel for nn_AudioVisualModel loss.

Strategy (8 NeuronCores, data-parallel over the VISUAL batch y):
  - Each core owns 3 of the 24 visual batches (4704 of 37632 visual rows,
    14.5MB) and replicates the full audio matrix (1200 rows, 3.7MB) --
    ~18MB HBM traffic per core vs ~119MB for audio-sharding.
  - Audio rows pad 1200->1280 (10 M-tiles of 128, 6.7% pad waste).
  - bf16 PE matmul produces sims per (M-tile, 1568-col group) in PSUM;
    stage copies to a SBUF bf16 slab rotate over act/gpsimd/vector.
  - Reductions exploit DVE fast modes (tensor_scalar 4x, tensor_tensor
    2x; the fused scalar_tensor_tensor runs 1x so it is avoided on the
    hot path):
      sq   = s*s                      (tt 2x)
      S_all, edge sums = ts+accum(sq) (4x)
      prod = s[t+1]*s[t]              (tt 2x, shifted views, per y)
      S_cross = ts+accum(prod)        (4x)
      mneg = min(s,0)                 (ts 4x)
      S_nonneg = Square(mneg)+accum   (act)
      max over Nv: tt-max halvings 196->98->49 (2x) + 49-wide reduce
    temporal loss = 2*S_all - S_t0 - S_t7 - 2*S_cross (exact identity;
    the reference's clamp at -20 is a no-op when 1/temp <= 20 -- slow
    path otherwise).
  - Visual norms on gpsimd, audio norms/muls on act, visual muls DVE 4x.
  - Device outputs per core: (3, 24) clip-sim partials and (128, 10, 8)
    per-partition reg partial sums. Host does the 576-element InfoNCE
    and final scalar assembly in f64.
"""

import math
import sys

import numpy as np

sys.path.insert(0, "/opt/trn_rl_repo")

import concourse.bass as bass
import concourse.tile as tile
from concourse import bacc, mybir
from concourse import masks as bass_masks
from concourse.bass_utils import run_bass_kernel_spmd

# Problem shapes (hardcoded per contract).
B, Na, T, Nv, D = 24, 50, 8, 196, 768
NCORES = 8
YPC = B // NCORES               # visual batches per core = 3
JY = T * Nv                     # visual rows per y = 1568
JC = YPC * JY                   # visual rows per core = 4704
AR = B * Na                     # audio rows total = 1200
MT = 10                         # audio M-tiles (1280 padded)
ABLK = 10                       # audio 128-blocks (9 full + 48)
VBLK = 37                       # visual 128-blocks (36 full + 96)
KC = D // 128                   # contraction chunks = 6
NCH = 2 * Nv                    # matmul N chunk = 392 (fits a PSUM bank)
NGRP = 3                        # chunk groups per M-tile (4 chunks each)
ACC_K = 16                      # accumulator slots per M-tile (5 per y)
EPS = 1e-12

_CACHE = {}


def _build(temp: float, thr: float):
    f32 = mybir.dt.float32
    bf16 = mybir.dt.bfloat16
    Alu = mybir.AluOpType
    Act = mybir.ActivationFunctionType

    fast_nonneg = (1.0 / temp) <= 20.0

    nc = bacc.Bacc(
        "TRN2",
        target_bir_lowering=False,
        debug=False,
        enable_asserts=False,
        num_devices=NCORES,
    )

    a_in = nc.dram_tensor("a", [AR, D], f32, kind="ExternalInput").ap()
    v_in = nc.dram_tensor("v", [JC, D], f32, kind="ExternalInput").ap()
    ind_in = nc.dram_tensor("ind", [128, MT * B], f32, kind="ExternalInput").ap()
    clip_out = nc.dram_tensor("clip", [YPC, B], f32, kind="ExternalOutput").ap()
    acc_out = nc.dram_tensor("acc", [128, MT * ACC_K], f32, kind="ExternalOutput").ap()

    with tile.TileContext(nc) as tc:
        from contextlib import ExitStack

        ctx = ExitStack()
        with ctx:
            singles = ctx.enter_context(tc.tile_pool(name="singles", bufs=1))
            tiny = ctx.enter_context(tc.tile_pool(name="tiny", bufs=2))

            ident = singles.tile([128, 128], bf16)
            bass_masks.make_identity(nc, ident[:])

            indt = singles.tile([128, MT, B], f32)
            nc.sync.dma_start(out=indt[:], in_=ind_in)

            acc = singles.tile([128, MT, ACC_K], f32)
            nc.vector.memset(acc[:], 0.0)
            maxv_all = singles.tile([128, MT, YPC * T], f32)

            # aT: 16*normalized-audio^T as fp8 pairs (128, KC/2, 2, 1280);
            # vt: 16*visual^T/temp likewise. The PE matmul runs fp8
            # DoubleRow (2 k-chunks per instruction, 0.5 cycles/row); the
            # stage copies undo the 16*16 scaling.
            fp8 = mybir.dt.float8e4
            aT = singles.tile([128, KC // 2, 2, MT * 128], fp8)
            nc.vector.memset(aT[:, :, :, AR:], 0.0)
            vt = singles.tile([128, KC // 2, 2, JC], fp8)

            # ---------------- prep phase ----------------
            # Visual groups stream first (Pool/DVE norms alternate per
            # block); audio (bf16, tiny) is interleaved after group 1 so
            # aT is ready by the time the first matmuls can start.
            with ExitStack() as pctx:
                apool = pctx.enter_context(tc.tile_pool(name="ap", bufs=1))
                vbpool = pctx.enter_context(tc.tile_pool(name="vb", bufs=3))
                ptiny = pctx.enter_context(tc.tile_pool(name="pt", bufs=2))
                tppool = pctx.enter_context(
                    tc.tile_pool(name="tp", bufs=3, space="PSUM")
                )

                at = apool.tile([128, ABLK, D], bf16)
                ab = apool.tile([128, ABLK, D], bf16)
                n2a = ptiny.tile([128, ABLK], f32, tag="n2a", name="n2a")
                nrma = ptiny.tile([128, ABLK], f32, tag="nrma", name="nrma")
                rna = ptiny.tile([128, ABLK], f32, tag="rna", name="rna")
                n2v = ptiny.tile([128, VBLK], f32, tag="n2v", name="n2v")
                rnv = ptiny.tile([128, VBLK], f32, tag="rnv", name="rnv")
                ascr = apool.tile([128, D], bf16)
                nc.vector.memset(n2a[:], 1.0)
                nc.vector.memset(n2v[:], 1.0)

                def audio_prep():
                    nc.gpsimd.dma_start(
                        out=at[:, :9, :],
                        in_=a_in[: 9 * 128, :].rearrange("(b p) d -> p b d", p=128),
                    )
                    nc.gpsimd.dma_start(out=at[:48, 9, :], in_=a_in[9 * 128 :, :])
                    for b in range(ABLK):
                        P = 128 if b < 9 else 48
                        nc.vector.affine_mul_reduce(
                            out=ascr[:P],
                            accum_out=n2a[:P, b : b + 1],
                            in0=at[:P, b, :],
                            in1=at[:P, b, :],
                            scale=1.0,
                            bias=0.0,
                        )
                    # sqrt(n2/256) = ||a||/16  (fp8 operand scale)
                    nc.scalar.activation(
                        nrma[:], n2a[:], Act.Sqrt, scale=1.0 / 256.0
                    )
                    nc.vector.tensor_scalar_max(nrma[:], nrma[:], EPS)
                    nc.vector.reciprocal(rna[:], nrma[:])
                    for b in range(ABLK):
                        P = 128 if b < 9 else 48
                        nc.vector.tensor_scalar_mul(
                            ab[:P, b, :], at[:P, b, :], rna[:P, b : b + 1]
                        )
                    for k in range(KC):
                        for g0, nb in ((0, 8), (8, 2)):
                            pt = tppool.tile([128, 1024], bf16, tag="tp", name="pta")
                            for i in range(nb):
                                b = g0 + i
                                P = 128 if b < 9 else 48
                                nc.tensor.transpose(
                                    pt[:, i * 128 : i * 128 + P],
                                    ab[:P, b, k * 128 : (k + 1) * 128],
                                    ident[:P, :P],
                                )
                            width = (
                                nb * 128 if g0 + nb < ABLK else (nb - 1) * 128 + 48
                            )
                            dst = aT[:, k // 2, k % 2, g0 * 128 : g0 * 128 + width]
                            if k % 2 == 0:
                                nc.vector.tensor_copy(dst, pt[:, :width])
                            else:
                                nc.scalar.copy(dst, pt[:, :width])

                ei = 0
                for gi, (g0, nb) in enumerate(
                    ((0, 8), (8, 8), (16, 8), (24, 8), (32, 5))
                ):
                    vb = vbpool.tile([128, 8, D], bf16, tag="vb", name="vb")
                    nfull = nb if g0 + nb < VBLK else nb - 1
                    src = v_in[g0 * 128 : (g0 + nfull) * 128, :].rearrange(
                        "(b p) d -> p b d", p=128
                    )
                    nc.gpsimd.dma_start(out=vb[:, :nfull, :], in_=src)
                    if g0 + nb == VBLK:
                        nc.gpsimd.dma_start(
                            out=vb[:96, nb - 1, :],
                            in_=v_in[(g0 + nfull) * 128 :, :],
                        )
                    vscr = ptiny.tile([128, D], bf16, tag="vscr", name="vscr")
                    for i in range(nb):
                        b = g0 + i
                        P = 128 if b < VBLK - 1 else 96
                        if b % 2 == 0:
                            nc.scalar.activation(
                                vscr[:P],
                                vb[:P, i, :],
                                Act.Square,
                                accum_out=n2v[:P, b : b + 1],
                            )
                        else:
                            nc.vector.affine_mul_reduce(
                                out=vscr[:P],
                                accum_out=n2v[:P, b : b + 1],
                                in0=vb[:P, i, :],
                                in1=vb[:P, i, :],
                                scale=1.0,
                                bias=0.0,
                            )
                    # sqrt(n2 * temp^2/256) = ||v||*temp/16 (fp8 scale)
                    nrmv = ptiny.tile([128, 8], f32, tag="nrmv", name="nrmv")
                    nc.scalar.activation(
                        nrmv[:, :nb],
                        n2v[:, g0 : g0 + nb],
                        Act.Sqrt,
                        scale=float(temp * temp) / 256.0,
                    )
                    nc.vector.tensor_scalar_max(nrmv[:, :nb], nrmv[:, :nb], EPS)
                    nc.vector.reciprocal(rnv[:, g0 : g0 + nb], nrmv[:, :nb])
                    for i in range(nb):
                        b = g0 + i
                        P = 128 if b < VBLK - 1 else 96
                        nc.vector.tensor_scalar_mul(
                            vb[:P, i, :], vb[:P, i, :], rnv[:P, b : b + 1]
                        )
                    for k in range(KC):
                        pt = tppool.tile([128, 1024], bf16, tag="tp", name="ptv")
                        for i in range(nb):
                            b = g0 + i
                            P = 128 if b < VBLK - 1 else 96
                            nc.tensor.transpose(
                                pt[:, i * 128 : i * 128 + P],
                                vb[:P, i, k * 128 : (k + 1) * 128],
                                ident[:P, :P],
                            )
                        width = nb * 128 if g0 + nb < VBLK else (nb - 1) * 128 + 96
                        dst = vt[:, k // 2, k % 2, g0 * 128 : g0 * 128 + width]
                        if ei % 3 == 0:
                            nc.vector.tensor_copy(dst, pt[:, :width])
                        else:
                            nc.scalar.copy(dst, pt[:, :width])
                        ei += 1
                    if gi == 1:
                        audio_prep()

            # ---------------- main m-loop ----------------
            with ExitStack() as mctx:
                slabpool = mctx.enter_context(tc.tile_pool(name="slab", bufs=3))
                scrpool = mctx.enter_context(tc.tile_pool(name="scr", bufs=2))
                mnpool = mctx.enter_context(tc.tile_pool(name="mn", bufs=2))
                hpool = mctx.enter_context(tc.tile_pool(name="h", bufs=2))
                mmpool = mctx.enter_context(
                    tc.tile_pool(name="mm", bufs=2, space="PSUM")
                )
                for m in range(MT):
                    # slab holds 256*s (fp8 operand scale); all rescaling
                    # is folded into host sums / ind / the thr constant.
                    # Each 1568-col mm-group is exactly one y, so every
                    # reduction runs per (m, y) right after its stage --
                    # 3x finer pipelining than per-M-tile granularity.
                    slab = slabpool.tile([128, JC], bf16, tag="s", name="slab")
                    sy3 = slab.rearrange("p (y c) -> p y c", y=YPC)
                    scr = scrpool.tile([128, JC], bf16, tag="stt", name="sttscr")
                    mscr = scrpool.tile([128, JC], bf16, tag="ms", name="mscr")
                    for y in range(YPC):
                        ps = mmpool.tile([128, 4, 512], f32, tag="ps", name="ps")
                        for kk in range(KC // 2):
                            for c in range(4):
                                j0 = (y * 4 + c) * NCH
                                nc.tensor.matmul(
                                    ps[:, c, :NCH],
                                    lhsT=aT[:, kk, :, m * 128 : (m + 1) * 128],
                                    rhs=vt[:, kk, :, j0 : j0 + NCH],
                                    start=(kk == 0),
                                    stop=(kk == KC // 2 - 1),
                                    perf_mode=mybir.MatmulPerfMode.DoubleRow,
                                )
                        s_y = sy3[:, y, :]
                        dst = s_y.rearrange("p (c j) -> p c j", c=4)
                        src_ps = ps[:, :, :NCH]
                        if y == 0 or (y == 2 and m % 2 == 0):
                            nc.scalar.copy(dst, src_ps)
                        else:
                            nc.vector.tensor_copy(dst, src_ps)

                        base = y * 5
                        ycol = slice(y * JY, (y + 1) * JY)
                        # sum s^2 (act)
                        nc.scalar.activation(
                            mscr[:, ycol],
                            s_y,
                            Act.Square,
                            accum_out=acc[:, m, base : base + 1],
                        )
                        # edge sums (DVE affine_mul_reduce, small)
                        vsc2 = scrpool.tile([128, Nv], bf16, tag="e", name="escr")
                        nc.vector.affine_mul_reduce(
                            out=vsc2[:],
                            accum_out=acc[:, m, base + 3 : base + 4],
                            in0=s_y[:, :Nv],
                            in1=s_y[:, :Nv],
                            scale=1.0,
                            bias=0.0,
                        )
                        nc.vector.affine_mul_reduce(
                            out=vsc2[:],
                            accum_out=acc[:, m, base + 4 : base + 5],
                            in0=s_y[:, JY - Nv :],
                            in1=s_y[:, JY - Nv :],
                            scale=1.0,
                            bias=0.0,
                        )
                        # temporal cross: prod on gpsimd, 4x accum on DVE
                        prod = scr[:, y * (JY - Nv) : (y + 1) * (JY - Nv)]
                        nc.gpsimd.tensor_tensor(
                            out=prod,
                            in0=s_y[:, Nv:],
                            in1=s_y[:, : JY - Nv],
                            op=Alu.mult,
                        )
                        nc.vector.tensor_scalar(
                            out=prod,
                            in0=prod,
                            scalar1=0.0,
                            scalar2=0.0,
                            op0=Alu.add,
                            op1=Alu.add,
                            accum_out=acc[:, m, base + 2 : base + 3],
                        )
                        # nonneg: mneg = min(s,0) (ts 4x), square+accum (act)
                        mneg = mnpool.tile([128, JY], bf16, tag="mn", name="mneg")
                        if fast_nonneg:
                            nc.vector.tensor_scalar_min(mneg[:], s_y, 0.0)
                        else:
                            nc.vector.tensor_scalar(
                                out=mneg[:],
                                in0=s_y,
                                scalar1=0.0,
                                scalar2=-20.0,
                                op0=Alu.min,
                                op1=Alu.max,
                            )
                        nc.scalar.activation(
                            mscr[:, ycol],
                            mneg[:],
                            Act.Square,
                            accum_out=acc[:, m, base + 1 : base + 2],
                        )
                        # max over Nv: two tt-max halvings + 49-wide reduce
                        sv = s_y.rearrange("p (t v) -> p t v", v=Nv)
                        h1 = hpool.tile([128, T, 98], bf16, tag="h1", name="h1")
                        nc.vector.tensor_tensor(
                            out=h1[:],
                            in0=sv[:, :, :98],
                            in1=sv[:, :, 98:],
                            op=Alu.max,
                        )
                        h2 = hpool.tile([128, T, 49], bf16, tag="h2", name="h2")
                        nc.vector.tensor_tensor(
                            out=h2[:],
                            in0=h1[:, :, :49],
                            in1=h1[:, :, 49:],
                            op=Alu.max,
                        )
                        nc.vector.reduce_max(
                            maxv_all[:, m, y * T : (y + 1) * T],
                            h2[:],
                            axis=mybir.AxisListType.X,
                        )

            # ---------------- epilogue ----------------
            with ExitStack() as ectx:
                clpool = ectx.enter_context(
                    tc.tile_pool(name="cl", bufs=1, space="PSUM")
                )
                epool = ectx.enter_context(tc.tile_pool(name="ep", bufs=1))
                mask = epool.tile([128, MT, YPC, T], f32)
                # maxv is at 256x scale; compare against 256*thr
                nc.vector.tensor_scalar(
                    out=mask.rearrange("p m y t -> p (m y t)"),
                    in0=maxv_all.rearrange("p m g -> p (m g)"),
                    scalar1=thr * 256.0,
                    scalar2=None,
                    op0=Alu.is_ge,
                )
                msked = epool.tile([128, MT, YPC, T], f32)
                nc.vector.tensor_tensor(
                    out=msked.rearrange("p m y t -> p (m y t)"),
                    in0=maxv_all.rearrange("p m g -> p (m g)"),
                    in1=mask.rearrange("p m y t -> p (m y t)"),
                    op=Alu.mult,
                )
                counts = epool.tile([128, MT, YPC], f32, tag="cnt", name="counts")
                nc.vector.reduce_sum(counts[:], mask[:], axis=mybir.AxisListType.X)
                toksum = epool.tile([128, MT, YPC], f32, tag="tks", name="toksum")
                nc.vector.reduce_sum(toksum[:], msked[:], axis=mybir.AxisListType.X)
                nc.vector.tensor_scalar_max(counts[:], counts[:], 1.0)
                rcc = epool.tile([128, MT, YPC], f32, tag="rcc", name="rcc")
                nc.vector.reciprocal(rcc[:], counts[:])
                tok = epool.tile([128, MT, YPC], f32, tag="tok", name="tok")
                nc.vector.tensor_tensor(
                    out=tok.rearrange("p m y -> p (m y)"),
                    in0=toksum.rearrange("p m y -> p (m y)"),
                    in1=rcc.rearrange("p m y -> p (m y)"),
                    op=Alu.mult,
                )
                psc = clpool.tile([YPC, B], f32, name="psc")
                for m in range(MT):
                    nc.tensor.matmul(
                        psc[:, :],
                        lhsT=tok[:, m, :],
                        rhs=indt[:, m, :],
                        start=(m == 0),
                        stop=(m == MT - 1),
                    )
                cls = epool.tile([YPC, B], f32, tag="cls", name="cls")
                nc.vector.tensor_copy(cls[:], psc[:])
                nc.sync.dma_start(out=clip_out[:, :], in_=cls[:])
                nc.sync.dma_start(
                    out=acc_out[:, :], in_=acc.rearrange("p m k -> p (m k)")
                )

    nc.compile()
    return nc


def _make_ind():
    # 1/(Na*256): folds the fp8 256x operand scale out of the clip sims
    ind = np.zeros((128, MT, B), dtype=np.float32)
    for m in range(MT):
        for p in range(128):
            row = m * 128 + p
            if row < AR:
                ind[p, m, row // Na] = 1.0 / (Na * 256.0)
    return ind.reshape(128, MT * B)


def kernel(audio_feats, visual_feats, temperature, threshold):
    temp = float(np.asarray(temperature))
    thr_in = float(np.asarray(threshold))
    thr = 1.0 / (1.0 + math.exp(-thr_in))  # sigmoid

    key = (temp, thr_in)
    if key not in _CACHE:
        _CACHE[key] = _build(temp, thr)
    nc = _CACHE[key]

    a = np.ascontiguousarray(
        np.asarray(audio_feats, dtype=np.float32).reshape(AR, D)
    )
    v = np.asarray(visual_feats, dtype=np.float32).reshape(B * JY, D)
    ind = _make_ind()

    in_maps = []
    for c in range(NCORES):
        in_maps.append(
            {
                "a": a,
                "v": np.ascontiguousarray(v[c * JC : (c + 1) * JC]),
                "ind": ind,
            }
        )

    res = run_bass_kernel_spmd(nc, in_maps, core_ids=list(range(NCORES)))
    outs = res.results

    clip = np.zeros((B, B), dtype=np.float64)
    s_all = s_nn = s_cr = s_e0 = s_e7 = 0.0
    for c in range(NCORES):
        co = outs[c]["clip"].astype(np.float64)  # (YPC, B): [y_local, x]
        for yl in range(YPC):
            clip[:, c * YPC + yl] = co[yl, :]
        # device sums are at (256*s)^2 scale; slots are per (m, y)
        ac = outs[c]["acc"].astype(np.float64).reshape(128, MT, ACC_K) / 65536.0
        for yl in range(YPC):
            s_all += ac[:, :, yl * 5 + 0].sum()
            s_nn += ac[:, :, yl * 5 + 1].sum()
            s_cr += ac[:, :, yl * 5 + 2].sum()
            s_e0 += ac[:, :, yl * 5 + 3].sum()
            s_e7 += ac[:, :, yl * 5 + 4].sum()

    def logsumexp(m, axis):
        mx = m.max(axis=axis, keepdims=True)
        return mx + np.log(np.exp(m - mx).sum(axis=axis, keepdims=True))

    diag = np.arange(B)
    lsm1 = clip - logsumexp(clip, 1)
    lsm0 = clip - logsumexp(clip, 0)
    contrastive = -(lsm1[diag, diag] + lsm0[diag, diag]).mean() / 2.0

    l_nonneg = s_nn / (B * B * Na * T * Nv)
    td_sum = 2.0 * s_all - s_e0 - s_e7 - 2.0 * s_cr
    l_temporal = td_sum / (B * B * Na * (T - 1) * Nv)
    log_t = math.log(temp)
    temp_low = max(math.log(2.3) - log_t, 0.0) ** 3
    temp_high = max(log_t - math.log(4.0), 0.0) ** 3
    reg = 0.15 * l_nonneg + 8.0 * (temp_low + temp_high) + 0.01 * l_temporal

    return np.float32(contrastive + reg)


# revision 26
# speedup vs baseline: 2.7877x; 1.0419x over previous
"""Trainium2 Bass kernel for nn_AudioVisualModel loss.

Strategy (8 NeuronCores, data-parallel over the VISUAL batch y):
  - Each core owns 3 of the 24 visual batches (4704 of 37632 visual rows,
    14.5MB) and replicates the full audio matrix (1200 rows, 3.7MB) --
    ~18MB HBM traffic per core vs ~119MB for audio-sharding.
  - Audio rows pad 1200->1280 (10 M-tiles of 128, 6.7% pad waste).
  - bf16 PE matmul produces sims per (M-tile, 1568-col group) in PSUM;
    stage copies to a SBUF bf16 slab rotate over act/gpsimd/vector.
  - Reductions exploit DVE fast modes (tensor_scalar 4x, tensor_tensor
    2x; the fused scalar_tensor_tensor runs 1x so it is avoided on the
    hot path):
      sq   = s*s                      (tt 2x)
      S_all, edge sums = ts+accum(sq) (4x)
      prod = s[t+1]*s[t]              (tt 2x, shifted views, per y)
      S_cross = ts+accum(prod)        (4x)
      mneg = min(s,0)                 (ts 4x)
      S_nonneg = Square(mneg)+accum   (act)
      max over Nv: tt-max halvings 196->98->49 (2x) + 49-wide reduce
    temporal loss = 2*S_all - S_t0 - S_t7 - 2*S_cross (exact identity;
    the reference's clamp at -20 is a no-op when 1/temp <= 20 -- slow
    path otherwise).
  - Visual norms on gpsimd, audio norms/muls on act, visual muls DVE 4x.
  - Device outputs per core: (3, 24) clip-sim partials and (128, 10, 8)
    per-partition reg partial sums. Host does the 576-element InfoNCE
    and final scalar assembly in f64.
"""

import math
import sys

import numpy as np

sys.path.insert(0, "/opt/trn_rl_repo")

import concourse.bass as bass
import concourse.tile as tile
from concourse import bacc, mybir
from concourse import masks as bass_masks
from concourse.bass_utils import run_bass_kernel_spmd

# Problem shapes (hardcoded per contract).
B, Na, T, Nv, D = 24, 50, 8, 196, 768
NCORES = 8
YPC = B // NCORES               # visual batches per core = 3
JY = T * Nv                     # visual rows per y = 1568
JC = YPC * JY                   # visual rows per core = 4704
AR = B * Na                     # audio rows total = 1200
MT = 10                         # audio M-tiles (1280 padded)
ABLK = 10                       # audio 128-blocks (9 full + 48)
VBLK = 37                       # visual 128-blocks (36 full + 96)
KC = D // 128                   # contraction chunks = 6
NCH = 2 * Nv                    # matmul N chunk = 392 (fits a PSUM bank)
NGRP = 3                        # chunk groups per M-tile (4 chunks each)
ACC_K = 16                      # accumulator slots per M-tile (5 per y)
EPS = 1e-12

_CACHE = {}


def _build(temp: float, thr: float):
    f32 = mybir.dt.float32
    bf16 = mybir.dt.bfloat16
    Alu = mybir.AluOpType
    Act = mybir.ActivationFunctionType

    fast_nonneg = (1.0 / temp) <= 20.0

    nc = bacc.Bacc(
        "TRN2",
        target_bir_lowering=False,
        debug=False,
        enable_asserts=False,
        num_devices=NCORES,
    )

    a_in = nc.dram_tensor("a", [AR, D], f32, kind="ExternalInput").ap()
    v_in = nc.dram_tensor("v", [JC, D], f32, kind="ExternalInput").ap()
    ind_in = nc.dram_tensor("ind", [128, MT * B], f32, kind="ExternalInput").ap()
    clip_out = nc.dram_tensor("clip", [YPC, B], f32, kind="ExternalOutput").ap()
    acc_out = nc.dram_tensor("acc", [128, MT * ACC_K], f32, kind="ExternalOutput").ap()

    with tile.TileContext(nc) as tc:
        from contextlib import ExitStack

        ctx = ExitStack()
        with ctx:
            singles = ctx.enter_context(tc.tile_pool(name="singles", bufs=1))
            tiny = ctx.enter_context(tc.tile_pool(name="tiny", bufs=2))

            ident = singles.tile([128, 128], bf16)
            bass_masks.make_identity(nc, ident[:])

            indt = singles.tile([128, MT, B], f32)
            nc.sync.dma_start(out=indt[:], in_=ind_in)

            acc = singles.tile([128, MT, ACC_K], f32)
            nc.vector.memset(acc[:], 0.0)
            maxv_all = singles.tile([128, MT, YPC * T], f32)

            # aT: 16*normalized-audio^T as fp8 pairs (128, KC/2, 2, 1280);
            # vt: 16*visual^T/temp likewise. The PE matmul runs fp8
            # DoubleRow (2 k-chunks per instruction, 0.5 cycles/row); the
            # stage copies undo the 16*16 scaling.
            fp8 = mybir.dt.float8e4
            aT = singles.tile([128, KC // 2, 2, MT * 128], fp8)
            nc.vector.memset(aT[:, :, :, AR:], 0.0)
            vt = singles.tile([128, KC // 2, 2, JC], fp8)

            # ---------------- prep phase ----------------
            # Visual groups stream first (Pool/DVE norms alternate per
            # block); audio (bf16, tiny) is interleaved after group 1 so
            # aT is ready by the time the first matmuls can start.
            with ExitStack() as pctx:
                apool = pctx.enter_context(tc.tile_pool(name="ap", bufs=1))
                vbpool = pctx.enter_context(tc.tile_pool(name="vb", bufs=3))
                ptiny = pctx.enter_context(tc.tile_pool(name="pt", bufs=2))
                tppool = pctx.enter_context(
                    tc.tile_pool(name="tp", bufs=3, space="PSUM")
                )

                at = apool.tile([128, ABLK, D], bf16)
                ab = apool.tile([128, ABLK, D], bf16)
                n2a = ptiny.tile([128, ABLK], f32, tag="n2a", name="n2a")
                nrma = ptiny.tile([128, ABLK], f32, tag="nrma", name="nrma")
                rna = ptiny.tile([128, ABLK], f32, tag="rna", name="rna")
                n2v = ptiny.tile([128, VBLK], f32, tag="n2v", name="n2v")
                rnv = ptiny.tile([128, VBLK], f32, tag="rnv", name="rnv")
                ascr = apool.tile([128, D], bf16)
                nc.vector.memset(n2a[:], 1.0)
                nc.vector.memset(n2v[:], 1.0)

                def audio_prep():
                    nc.gpsimd.dma_start(
                        out=at[:, :9, :],
                        in_=a_in[: 9 * 128, :].rearrange("(b p) d -> p b d", p=128),
                    )
                    nc.gpsimd.dma_start(out=at[:48, 9, :], in_=a_in[9 * 128 :, :])
                    for b in range(ABLK):
                        P = 128 if b < 9 else 48
                        nc.vector.affine_mul_reduce(
                            out=ascr[:P],
                            accum_out=n2a[:P, b : b + 1],
                            in0=at[:P, b, :],
                            in1=at[:P, b, :],
                            scale=1.0,
                            bias=0.0,
                        )
                    # sqrt(n2/256) = ||a||/16  (fp8 operand scale)
                    nc.scalar.activation(
                        nrma[:], n2a[:], Act.Sqrt, scale=1.0 / 256.0
                    )
                    nc.vector.tensor_scalar_max(nrma[:], nrma[:], EPS)
                    nc.vector.reciprocal(rna[:], nrma[:])
                    for b in range(ABLK):
                        P = 128 if b < 9 else 48
                        nc.vector.tensor_scalar_mul(
                            ab[:P, b, :], at[:P, b, :], rna[:P, b : b + 1]
                        )
                    for k in range(KC):
                        for g0, nb in ((0, 8), (8, 2)):
                            pt = tppool.tile([128, 1024], bf16, tag="tp", name="pta")
                            for i in range(nb):
                                b = g0 + i
                                P = 128 if b < 9 else 48
                                nc.tensor.transpose(
                                    pt[:, i * 128 : i * 128 + P],
                                    ab[:P, b, k * 128 : (k + 1) * 128],
                                    ident[:P, :P],
                                )
                            width = (
                                nb * 128 if g0 + nb < ABLK else (nb - 1) * 128 + 48
                            )
                            dst = aT[:, k // 2, k % 2, g0 * 128 : g0 * 128 + width]
                            if k % 2 == 0:
                                nc.vector.tensor_copy(dst, pt[:, :width])
                            else:
                                nc.scalar.copy(dst, pt[:, :width])

                ei = 0
                for gi, (g0, nb) in enumerate(
                    ((0, 4), (4, 4), (8, 8), (16, 8), (24, 8), (32, 5))
                ):
                    vb = vbpool.tile([128, 8, D], bf16, tag="vb", name="vb")
                    nfull = nb if g0 + nb < VBLK else nb - 1
                    src = v_in[g0 * 128 : (g0 + nfull) * 128, :].rearrange(
                        "(b p) d -> p b d", p=128
                    )
                    nc.gpsimd.dma_start(out=vb[:, :nfull, :], in_=src)
                    if g0 + nb == VBLK:
                        nc.gpsimd.dma_start(
                            out=vb[:96, nb - 1, :],
                            in_=v_in[(g0 + nfull) * 128 :, :],
                        )
                    vscr = ptiny.tile([128, D], bf16, tag="vscr", name="vscr")
                    for i in range(nb):
                        b = g0 + i
                        P = 128 if b < VBLK - 1 else 96
                        if b % 2 == 0:
                            nc.scalar.activation(
                                vscr[:P],
                                vb[:P, i, :],
                                Act.Square,
                                accum_out=n2v[:P, b : b + 1],
                            )
                        else:
                            nc.vector.affine_mul_reduce(
                                out=vscr[:P],
                                accum_out=n2v[:P, b : b + 1],
                                in0=vb[:P, i, :],
                                in1=vb[:P, i, :],
                                scale=1.0,
                                bias=0.0,
                            )
                    # sqrt(n2 * temp^2/256) = ||v||*temp/16 (fp8 scale)
                    nrmv = ptiny.tile([128, 8], f32, tag="nrmv", name="nrmv")
                    nc.scalar.activation(
                        nrmv[:, :nb],
                        n2v[:, g0 : g0 + nb],
                        Act.Sqrt,
                        scale=float(temp * temp) / 256.0,
                    )
                    nc.vector.tensor_scalar_max(nrmv[:, :nb], nrmv[:, :nb], EPS)
                    nc.vector.reciprocal(rnv[:, g0 : g0 + nb], nrmv[:, :nb])
                    for i in range(nb):
                        b = g0 + i
                        P = 128 if b < VBLK - 1 else 96
                        nc.vector.tensor_scalar_mul(
                            vb[:P, i, :], vb[:P, i, :], rnv[:P, b : b + 1]
                        )
                    for k in range(KC):
                        pt = tppool.tile([128, 1024], bf16, tag="tp", name="ptv")
                        for i in range(nb):
                            b = g0 + i
                            P = 128 if b < VBLK - 1 else 96
                            nc.tensor.transpose(
                                pt[:, i * 128 : i * 128 + P],
                                vb[:P, i, k * 128 : (k + 1) * 128],
                                ident[:P, :P],
                            )
                        width = nb * 128 if g0 + nb < VBLK else (nb - 1) * 128 + 96
                        dst = vt[:, k // 2, k % 2, g0 * 128 : g0 * 128 + width]
                        if ei % 3 == 0:
                            nc.vector.tensor_copy(dst, pt[:, :width])
                        else:
                            nc.scalar.copy(dst, pt[:, :width])
                        ei += 1
                    if gi == 1:
                        audio_prep()

            # ---------------- main m-loop ----------------
            with ExitStack() as mctx:
                slabpool = mctx.enter_context(tc.tile_pool(name="slab", bufs=3))
                scrpool = mctx.enter_context(tc.tile_pool(name="scr", bufs=2))
                mnpool = mctx.enter_context(tc.tile_pool(name="mn", bufs=2))
                hpool = mctx.enter_context(tc.tile_pool(name="h", bufs=2))
                mmpool = mctx.enter_context(
                    tc.tile_pool(name="mm", bufs=2, space="PSUM")
                )
                for m in range(MT):
                    # slab holds 256*s (fp8 operand scale); all rescaling
                    # is folded into host sums / ind / the thr constant.
                    # Each 1568-col mm-group is exactly one y, so every
                    # reduction runs per (m, y) right after its stage --
                    # 3x finer pipelining than per-M-tile granularity.
                    slab = slabpool.tile([128, JC], bf16, tag="s", name="slab")
                    sy3 = slab.rearrange("p (y c) -> p y c", y=YPC)
                    scr = scrpool.tile([128, JC], bf16, tag="stt", name="sttscr")
                    mscr = scrpool.tile([128, JC], bf16, tag="ms", name="mscr")
                    for y in range(YPC):
                        ps = mmpool.tile([128, 4, 512], f32, tag="ps", name="ps")
                        for kk in range(KC // 2):
                            for c in range(4):
                                j0 = (y * 4 + c) * NCH
                                nc.tensor.matmul(
                                    ps[:, c, :NCH],
                                    lhsT=aT[:, kk, :, m * 128 : (m + 1) * 128],
                                    rhs=vt[:, kk, :, j0 : j0 + NCH],
                                    start=(kk == 0),
                                    stop=(kk == KC // 2 - 1),
                                    perf_mode=mybir.MatmulPerfMode.DoubleRow,
                                )
                        s_y = sy3[:, y, :]
                        dst = s_y.rearrange("p (c j) -> p c j", c=4)
                        src_ps = ps[:, :, :NCH]
                        if y == 0 or (y == 2 and m % 2 == 0):
                            nc.scalar.copy(dst, src_ps)
                        else:
                            nc.vector.tensor_copy(dst, src_ps)

                        base = y * 5
                        ycol = slice(y * JY, (y + 1) * JY)
                        # edge sums (DVE affine_mul_reduce, small)
                        vsc2 = scrpool.tile([128, Nv], bf16, tag="e", name="escr")
                        nc.vector.affine_mul_reduce(
                            out=vsc2[:],
                            accum_out=acc[:, m, base + 3 : base + 4],
                            in0=s_y[:, :Nv],
                            in1=s_y[:, :Nv],
                            scale=1.0,
                            bias=0.0,
                        )
                        nc.vector.affine_mul_reduce(
                            out=vsc2[:],
                            accum_out=acc[:, m, base + 4 : base + 5],
                            in0=s_y[:, JY - Nv :],
                            in1=s_y[:, JY - Nv :],
                            scale=1.0,
                            bias=0.0,
                        )
                        # temporal cross: prod on gpsimd, 4x accum on DVE
                        prod = scr[:, y * (JY - Nv) : (y + 1) * (JY - Nv)]
                        nc.gpsimd.tensor_tensor(
                            out=prod,
                            in0=s_y[:, Nv:],
                            in1=s_y[:, : JY - Nv],
                            op=Alu.mult,
                        )
                        nc.vector.tensor_scalar(
                            out=prod,
                            in0=prod,
                            scalar1=0.0,
                            scalar2=0.0,
                            op0=Alu.add,
                            op1=Alu.add,
                            accum_out=acc[:, m, base + 2 : base + 3],
                        )
                        # nonneg: mneg = min(s,0) (ts 4x), square+accum (act)
                        mneg = mnpool.tile([128, JY], bf16, tag="mn", name="mneg")
                        if fast_nonneg:
                            nc.vector.tensor_scalar_min(mneg[:], s_y, 0.0)
                        else:
                            nc.vector.tensor_scalar(
                                out=mneg[:],
                                in0=s_y,
                                scalar1=0.0,
                                scalar2=-20.0,
                                op0=Alu.min,
                                op1=Alu.max,
                            )
                        nc.scalar.activation(
                            mscr[:, ycol],
                            mneg[:],
                            Act.Square,
                            accum_out=acc[:, m, base + 1 : base + 2],
                        )
                        # max over Nv: two tt-max halvings + 49-wide reduce
                        sv = s_y.rearrange("p (t v) -> p t v", v=Nv)
                        h1 = hpool.tile([128, T, 98], bf16, tag="h1", name="h1")
                        nc.vector.tensor_tensor(
                            out=h1[:],
                            in0=sv[:, :, :98],
                            in1=sv[:, :, 98:],
                            op=Alu.max,
                        )
                        h2 = hpool.tile([128, T, 49], bf16, tag="h2", name="h2")
                        nc.vector.tensor_tensor(
                            out=h2[:],
                            in0=h1[:, :, :49],
                            in1=h1[:, :, 49:],
                            op=Alu.max,
                        )
                        nc.vector.reduce_max(
                            maxv_all[:, m, y * T : (y + 1) * T],
                            h2[:],
                            axis=mybir.AxisListType.X,
                        )
                    # sum s^2 over the whole M-tile (act, one op; slot 0)
                    nc.scalar.activation(
                        mscr[:],
                        slab[:],
                        Act.Square,
                        accum_out=acc[:, m, 0:1],
                    )

            # ---------------- epilogue ----------------
            with ExitStack() as ectx:
                clpool = ectx.enter_context(
                    tc.tile_pool(name="cl", bufs=1, space="PSUM")
                )
                epool = ectx.enter_context(tc.tile_pool(name="ep", bufs=1))
                mask = epool.tile([128, MT, YPC, T], f32)
                # maxv is at 256x scale; compare against 256*thr
                nc.vector.tensor_scalar(
                    out=mask.rearrange("p m y t -> p (m y t)"),
                    in0=maxv_all.rearrange("p m g -> p (m g)"),
                    scalar1=thr * 256.0,
                    scalar2=None,
                    op0=Alu.is_ge,
                )
                msked = epool.tile([128, MT, YPC, T], f32)
                nc.vector.tensor_tensor(
                    out=msked.rearrange("p m y t -> p (m y t)"),
                    in0=maxv_all.rearrange("p m g -> p (m g)"),
                    in1=mask.rearrange("p m y t -> p (m y t)"),
                    op=Alu.mult,
                )
                counts = epool.tile([128, MT, YPC], f32, tag="cnt", name="counts")
                nc.vector.reduce_sum(counts[:], mask[:], axis=mybir.AxisListType.X)
                toksum = epool.tile([128, MT, YPC], f32, tag="tks", name="toksum")
                nc.vector.reduce_sum(toksum[:], msked[:], axis=mybir.AxisListType.X)
                nc.vector.tensor_scalar_max(counts[:], counts[:], 1.0)
                rcc = epool.tile([128, MT, YPC], f32, tag="rcc", name="rcc")
                nc.vector.reciprocal(rcc[:], counts[:])
                tok = epool.tile([128, MT, YPC], f32, tag="tok", name="tok")
                nc.vector.tensor_tensor(
                    out=tok.rearrange("p m y -> p (m y)"),
                    in0=toksum.rearrange("p m y -> p (m y)"),
                    in1=rcc.rearrange("p m y -> p (m y)"),
                    op=Alu.mult,
                )
                psc = clpool.tile([YPC, B], f32, name="psc")
                for m in range(MT):
                    nc.tensor.matmul(
                        psc[:, :],
                        lhsT=tok[:, m, :],
                        rhs=indt[:, m, :],
                        start=(m == 0),
                        stop=(m == MT - 1),
                    )
                cls = epool.tile([YPC, B], f32, tag="cls", name="cls")
                nc.vector.tensor_copy(cls[:], psc[:])
                nc.sync.dma_start(out=clip_out[:, :], in_=cls[:])
                nc.sync.dma_start(
                    out=acc_out[:, :], in_=acc.rearrange("p m k -> p (m k)")
                )

    nc.compile()
    return nc


def _make_ind():
    # 1/(Na*256): folds the fp8 256x operand scale out of the clip sims
    ind = np.zeros((128, MT, B), dtype=np.float32)
    for m in range(MT):
        for p in range(128):
            row = m * 128 + p
            if row < AR:
                ind[p, m, row // Na] = 1.0 / (Na * 256.0)
    return ind.reshape(128, MT * B)


def kernel(audio_feats, visual_feats, temperature, threshold):
    temp = float(np.asarray(temperature))
    thr_in = float(np.asarray(threshold))
    thr = 1.0 / (1.0 + math.exp(-thr_in))  # sigmoid

    key = (temp, thr_in)
    if key not in _CACHE:
        _CACHE[key] = _build(temp, thr)
    nc = _CACHE[key]

    a = np.ascontiguousarray(
        np.asarray(audio_feats, dtype=np.float32).reshape(AR, D)
    )
    v = np.asarray(visual_feats, dtype=np.float32).reshape(B * JY, D)
    ind = _make_ind()

    in_maps = []
    for c in range(NCORES):
        in_maps.append(
            {
                "a": a,
                "v": np.ascontiguousarray(v[c * JC : (c + 1) * JC]),
                "ind": ind,
            }
        )

    res = run_bass_kernel_spmd(nc, in_maps, core_ids=list(range(NCORES)))
    outs = res.results

    clip = np.zeros((B, B), dtype=np.float64)
    s_all = s_nn = s_cr = s_e0 = s_e7 = 0.0
    for c in range(NCORES):
        co = outs[c]["clip"].astype(np.float64)  # (YPC, B): [y_local, x]
        for yl in range(YPC):
            clip[:, c * YPC + yl] = co[yl, :]
        # device sums are at (256*s)^2 scale; slots are per (m, y)
        ac = outs[c]["acc"].astype(np.float64).reshape(128, MT, ACC_K) / 65536.0
        for yl in range(YPC):
            s_all += ac[:, :, yl * 5 + 0].sum()
            s_nn += ac[:, :, yl * 5 + 1].sum()
            s_cr += ac[:, :, yl * 5 + 2].sum()
            s_e0 += ac[:, :, yl * 5 + 3].sum()
            s_e7 += ac[:, :, yl * 5 + 4].sum()

    def logsumexp(m, axis):
        mx = m.max(axis=axis, keepdims=True)
        return mx + np.log(np.exp(m - mx).sum(axis=axis, keepdims=True))

    diag = np.arange(B)
    lsm1 = clip - logsumexp(clip, 1)
    lsm0 = clip - logsumexp(clip, 0)
    contrastive = -(lsm1[diag, diag] + lsm0[diag, diag]).mean() / 2.0

    l_nonneg = s_nn / (B * B * Na * T * Nv)
    td_sum = 2.0 * s_all - s_e0 - s_e7 - 2.0 * s_cr
    l_temporal = td_sum / (B * B * Na * (T - 1) * Nv)
    log_t = math.log(temp)
    temp_low = max(math.log(2.3) - log_t, 0.0) ** 3
    temp_high = max(log_t - math.log(4.0), 0.0) ** 3
    reg = 0.15 * l_nonneg + 8.0 * (temp_low + temp_high) + 0.01 * l_temporal

    return np.float32(contrastive + reg)
